# revision 27
# baseline (speedup 1.0000x reference)
"""BrokenBiasAttention Trainium2 kernel (8-core SPMD), v6.

Sharding: core c -> batch b=c//2, query-row-half r=c%2 (1024 of 2048 rows).

Structure:
  - Softmax elementwise work split across THREE engines per-tile:
      DVE tiles: Schraudolph one-touch tensor_add (psum f32 + i16 table
        -> i16 = bf16 bits of exp(s+b-20)); scores pre-scaled by
        A16=128/ln2 folded into host Wq.
      ACT+Pool tiles: ACT exp(s) (scale=1/A16 folds the prescale back
        out) -> bf16, then GpSimd(Pool) multiplies by the SAME i16
        table BITCAST as bf16 (= exp(b-20) Schraudolph bits). GpSimd
        cannot read PSUM and has no integer ALU, but does bf16 muls.
    Table bits carry a -7.34 mean correction so the Schraudolph
    sawtooth is zero-mean when exact-exp and approx tiles mix within
    one softmax row (rel err ~5.6e-3).
  - Merged score matmuls: one 32x128-lhsT matmul per head per kt
    (out [128 kpos, 512 q]) instead of four 32x32 volleys.
  - Projection matmuls ordered so same-rhs instructions are adjacent
    (PE input-stream sharing), single wide PSUM tiles, single-copy evac.
  - kT/qT/v in per-kt-chunk tiles so round 0 starts after ~1/4 of the
    projection work.
  - reciprocal_approx_fast epilogue at partition base 0 only (base-64
    invocations of the custom DVE op corrupt results on HW).
  - Table DMAs issued on the sync queue AFTER the x/w loads: the queue
    FIFO guarantees 8.6MB of table traffic cannot delay them.
"""

import math
import sys

import numpy as np

if "/opt/trn_rl_repo" not in sys.path:
    sys.path.insert(0, "/opt/trn_rl_repo")

N = 2048
C = 256
NH = 8
HD = 32
B = 4
QR = 1024  # q rows per core
S_SHIFT = 20.0
A16 = 128.0 / math.log(2.0)
# -7.34 zero-means the Schraudolph sawtooth so exact-exp and approx tiles
# can mix within one softmax row
B16 = 127.0 * 128.0 - 7.34

# merged rounds: (pair_a, pair_b) per qc; pairs must differ mod 2 so the
# four heads map to distinct 32-row PE bands
ROUNDS = [(0, 3), (1, 2)]

# bisection knobs: _ALL_DVE routes every tile through the DVE add path;
# _MUL_ENGINE picks the engine for the exp-path bias multiply
_ALL_DVE = False
_MUL_ENGINE = "pool"

_NC = None


def _build_nc(dbg=False):
    import concourse.bass as bass
    import concourse.tile as tile
    from concourse import bacc, mybir
    from concourse.bass import ds, ts

    f32 = mybir.dt.float32
    bf16 = mybir.dt.bfloat16
    i16 = mybir.dt.int16
    EXP = mybir.ActivationFunctionType.Exp

    nc = bacc.Bacc(None, target_bir_lowering=False, debug=False)

    xT = nc.dram_tensor("xT", [C, N], bf16, kind="ExternalInput")
    xTq = nc.dram_tensor("xTq", [C, QR], bf16, kind="ExternalInput")
    Wq_d = nc.dram_tensor("Wq", [C, C], bf16, kind="ExternalInput")
    Wk_d = nc.dram_tensor("Wk", [C, C], bf16, kind="ExternalInput")
    Wv_d = nc.dram_tensor("Wv", [C, C], bf16, kind="ExternalInput")
    Wo_d = nc.dram_tensor("Wo", [C, C], bf16, kind="ExternalInput")
    schT_d = nc.dram_tensor("schT", [128, 4 * 8448], i16, kind="ExternalInput")
    out_d = nc.dram_tensor("out", [QR, C], f32, kind="ExternalOutput")

    with tile.TileContext(nc) as tc:
        with (
            tc.tile_pool(name="consts", bufs=1) as consts,
            tc.tile_pool(name="tbl", bufs=1) as tbl,
            tc.tile_pool(name="xp", bufs=1) as xp,
            tc.tile_pool(name="kqv", bufs=1) as kqv,
            tc.tile_pool(name="ep", bufs=16) as ep,
            tc.tile_pool(name="rp", bufs=4) as rp,
            tc.tile_pool(name="otp", bufs=2) as otp,
            tc.tile_pool(name="stp", bufs=2) as stp,
            tc.tile_pool(name="spsum", bufs=3, space="PSUM") as spsum,
            tc.tile_pool(name="apsum", bufs=2, space="PSUM") as apsum,
        ):
            sch_sb = tbl.tile([128, 4 * 8448], i16, tag="sch")
            sch_view = sch_sb.rearrange("p (h r f) -> p h r f", h=8, r=11, f=384)

            # ---- constants ----
            w_sb = {}
            for name, d in (("Wq", Wq_d), ("Wk", Wk_d), ("Wv", Wv_d), ("Wo", Wo_d)):
                t = consts.tile([128, 2, C], bf16, tag=f"w_{name}", name=f"w_{name}")
                nc.sync.dma_start(out=t, in_=d[:].rearrange("(ch p) n -> p ch n", p=128))
                w_sb[name] = t
            ones_sb = consts.tile([128, 32], bf16, tag="ones")
            nc.vector.memset(ones_sb, 1.0)

            # ---- projections (all bf16; q scale folded into host Wq) ----
            # chunked output tiles so the main loop can start early:
            #   kTc[half][jc]: [128, 512] covering kt = 4*jc .. 4*jc+3
            #   qTq[half][qc]: [128, 512]
            #   vc[jc]: [128, 4, C] covering kt = 4*jc .. 4*jc+3
            kTc = [[kqv.tile([128, 512], bf16, tag=f"kT{m}_{j}", name=f"kT{m}_{j}")
                    for j in range(4)] for m in range(2)]
            qTq = [[kqv.tile([128, 512], bf16, tag=f"qT{m}_{j}", name=f"qT{m}_{j}")
                    for j in range(2)] for m in range(2)]
            vc = [kqv.tile([128, 4, C], bf16, tag=f"v{j}", name=f"v{j}")
                  for j in range(4)]

            # ---- all input DMAs issued up front (x/w first, then tables on
            # the same sync queue: FIFO keeps table traffic behind the loads)
            xTq_r = xTq[:].rearrange("(ch p) n -> p ch n", p=128)
            xq = xp.tile([128, 2, 1024], bf16, tag="xq")
            nc.sync.dma_start(out=xq, in_=xTq_r)
            xT_r = xT[:].rearrange("(ch p) n -> p ch n", p=128)
            xcs = []
            for j in range(N // 512):
                xc = xp.tile([128, 2, 512], bf16, tag=f"xc{j}")
                nc.sync.dma_start(out=xc, in_=xT_r[:, :, ds(512 * j, 512)])
                xcs.append(xc)
            for g2t in (0, 3, 1, 2):
                nc.sync.dma_start(
                    out=sch_sb[:, ds(g2t * 8448, 8448)],
                    in_=schT_d[:, ds(g2t * 8448, 8448)],
                )

            def emit_q():
                for j in range(QR // 512):
                    ps = spsum.tile([128, 1024], f32, tag="s")
                    for ch in range(2):
                        for m in range(2):
                            nc.tensor.matmul(
                                ps[:, ts(m, 512)],
                                lhsT=w_sb["Wq"][:, ch, ts(m, 128)],
                                rhs=xq[:, ch, ds(512 * j, 512)],
                                start=(ch == 0),
                                stop=(ch == 1),
                            )
                    for m in range(2):
                        nc.scalar.copy(qTq[m][j][:], ps[:, ts(m, 512)])

            def emit_kv(j):
                xc = xcs[j]
                ps = spsum.tile([128, 1024], f32, tag="s")
                for ch in range(2):
                    for m in range(2):
                        nc.tensor.matmul(
                            ps[:, ts(m, 512)],
                            lhsT=w_sb["Wk"][:, ch, ts(m, 128)],
                            rhs=xc[:, ch, :],
                            start=(ch == 0),
                            stop=(ch == 1),
                        )
                for m in range(2):
                    nc.scalar.copy(kTc[m][j][:], ps[:, ts(m, 512)])
                psv = spsum.tile([128, 1024], f32, tag="s")
                for t in range(4):
                    for ch in range(2):
                        nc.tensor.matmul(
                            psv[:, ts(t, C)],
                            lhsT=xc[:, ch, ts(t, 128)],
                            rhs=w_sb["Wv"][:, ch, :],
                            start=(ch == 0),
                            stop=(ch == 1),
                        )
                nc.scalar.copy(vc[j][:], psv[:].rearrange("p (t c) -> p t c", t=4))

            # ---- main attention: merged-pair rounds ----
            oT_tiles = []
            for qc in range(2):
                oT = otp.tile([128, 2, 512], bf16, tag="oT", name=f"oT{qc}")
                oT_tiles.append(oT)

            def emit_wo(qc):
                oTw = oT_tiles[qc]
                fps = spsum.tile([128, 1024], f32, tag="s")
                for s in range(4):
                    for ch in range(2):
                        nc.tensor.matmul(
                            fps[:, ts(s, C)],
                            lhsT=oTw[:, ch, ts(s, 128)],
                            rhs=w_sb["Wo"][:, ch, :],
                            start=(ch == 0),
                            stop=(ch == 1),
                        )
                stage = stp.tile([128, 4, C], f32, tag="stage")
                nc.scalar.copy(stage, fps[:].rearrange("p (s c) -> p s c", s=4))
                nc.sync.dma_start(
                    out=out_d[ds(512 * qc, 512), :].rearrange(
                        "(s p) c -> p s c", p=128
                    ),
                    in_=stage,
                )

            def emit_av(P, kt):
                e_t, e_i16 = P["e"].pop(kt)
                for k in range(2):
                    h = 2 * P["g2"] + k
                    rhs = e_t[:, ts(k, 512)]
                    if e_i16:
                        rhs = rhs.bitcast(bf16)
                    nc.tensor.matmul(
                        P["acc"][ds(P["po_av"] + 32 * k, 32), :],
                        lhsT=vc[kt // 4][:, kt % 4, ds(32 * h, 32)],
                        rhs=rhs,
                        start=(kt == 0),
                        stop=(kt == 15),
                        tile_position=(0, P["po_av"] + 32 * k),
                        skip_group_check=True,
                    )
                    nc.tensor.matmul(
                        P["acc"][ds(P["po_rs"] + 32 * k, 32), :],
                        lhsT=ones_sb,
                        rhs=rhs,
                        start=(kt == 0),
                        stop=(kt == 15),
                        tile_position=(0, P["po_rs"] + 32 * k),
                        skip_group_check=True,
                    )

            def begin_round(round_idx, qc, g2a, g2b):
                parts = []
                for pi, g2 in enumerate((g2a, g2b)):
                    parts.append({
                        "g2": g2,
                        "pi": pi,
                        "po_av": 0 if g2 % 2 == 0 else 64,
                        "po_rs": 64 if g2 % 2 == 0 else 0,
                        "half": g2 // 2,
                        "acc": apsum.tile([128, 512], f32, tag="acc",
                                          name=f"acc{g2}_{qc}"),
                        "e": {},
                    })
                return {"idx": round_idx, "qc": qc, "parts": parts}

            def do_kts(R, kts):
                qc, parts = R["qc"], R["parts"]
                for kt in kts:
                    rdw0 = 2 * qc - (kt // 2) + 7
                    woff = 128 if kt % 2 == 0 else 0
                    sps = []
                    for P in parts:
                        s_ps = spsum.tile([128, 1024], f32, tag="s")
                        sps.append(s_ps)
                    # merged scores: one 32x128-lhsT matmul per head
                    # (out [128 kpos, 512 q]); part-major order so pair
                    # a's tiles never queue behind pair b's psum dep.
                    for P, s_ps in zip(parts, sps):
                        for k in range(2):
                            h = 2 * P["g2"] + k
                            i = h % 4
                            nc.tensor.matmul(
                                s_ps[:, ts(k, 512)],
                                lhsT=kTc[P["half"]][kt // 4][
                                    ds(32 * i, 32),
                                    ds(128 * (kt % 4), 128),
                                ],
                                rhs=qTq[P["half"]][qc][ds(32 * i, 32), :],
                                start=True,
                                stop=True,
                                tile_position=(32 * i, 0),
                                skip_group_check=True,
                            )
                    # consumers: DVE Schraudolph-adds straight from PSUM
                    # for ~47% of tiles; the rest ACT exp(s) then multiply
                    # by the bitcast table on GpSimd (slow Q7 software,
                    # ~2.1us/tile -- kept off the round-tail kts so it
                    # never gates the epilogue) or DVE.
                    for P, s_ps in zip(parts, sps):
                        dve = _ALL_DVE or (P["pi"] == 1 and kt not in (5, 11))
                        bt4 = sch_view[
                            :, ds(2 * P["g2"], 2), ds(rdw0, 2),
                            ds(woff, 256)
                        ]
                        if dve:
                            e_sb = ep.tile([128, 1024], i16, tag="e")
                            e4 = e_sb.rearrange("p (k jj f) -> p k jj f",
                                                k=2, jj=2)
                            s4 = s_ps.rearrange("p (k jj f) -> p k jj f",
                                                k=2, jj=2)
                            nc.vector.tensor_add(e4, s4, bt4)
                            P["e"][kt] = (e_sb, True)
                        else:
                            x_sb = ep.tile([128, 1024], bf16, tag="e")
                            nc.scalar.activation(x_sb, s_ps, EXP,
                                                 scale=1.0 / A16)
                            e_sb = ep.tile([128, 1024], bf16, tag="e")
                            e4 = e_sb.rearrange("p (k jj f) -> p k jj f",
                                                k=2, jj=2)
                            x4 = x_sb.rearrange("p (k jj f) -> p k jj f",
                                                k=2, jj=2)
                            pool = (_MUL_ENGINE == "pool"
                                    and P["pi"] == 0
                                    and kt in (0, 1, 2, 3, 4, 5, 6, 8, 9, 11))
                            meng = nc.gpsimd if pool else nc.vector
                            meng.tensor_mul(e4, x4, bt4.bitcast(bf16))
                            P["e"][kt] = (e_sb, False)
                    if kt >= 3 and kt % 2 == 1:
                        for P in parts:
                            emit_av(P, kt - 3)
                            emit_av(P, kt - 2)
                    if R["idx"] == 2 and kt == 6:
                        # Wo for qc=0 rides inside this round instead of
                        # stalling the qc boundary
                        emit_wo(0)

            def finish_round(R):
                # remaining AVs + epilogue, part-major so pair a's
                # normalization overlaps pair b's tail matmuls
                oT = oT_tiles[R["qc"]]
                for P in R["parts"]:
                    emit_av(P, 14)
                    emit_av(P, 15)
                    po_av, po_rs = P["po_av"], P["po_rs"]
                    acc = P["acc"]
                    recip = rp.tile([128, 512], f32, tag="recip")
                    rep = rp.tile([128, 512], f32, tag="rep")
                    if po_rs == 0:
                        nc.vector.reciprocal_approx_fast(
                            recip[ds(0, 64), :], acc[ds(0, 64), :]
                        )
                    else:
                        nc.scalar.copy(
                            rep[ds(64, 64), :], acc[ds(64, 64), :]
                        )
                        nc.sync.dma_start(
                            out=rep[ds(0, 64), :], in_=rep[ds(64, 64), :]
                        )
                        nc.vector.reciprocal_approx_fast(
                            recip[ds(0, 64), :], rep[ds(0, 64), :]
                        )
                    if po_av == 0:
                        nc.vector.tensor_mul(
                            oT[ds(0, 64), P["half"], :],
                            acc[ds(0, 64), :],
                            recip[ds(0, 64), :],
                        )
                    else:
                        nc.sync.dma_start(
                            out=rep[ds(64, 64), :], in_=recip[ds(0, 64), :]
                        )
                        nc.vector.tensor_mul(
                            oT[ds(64, 64), P["half"], :],
                            acc[ds(64, 64), :],
                            rep[ds(64, 64), :],
                        )

            # ---- schedule: Q + K/V chunk 0 up front, then round 0's kt
            # groups interleaved with the remaining K/V projection chunks
            # so the PE never idles on prologue DMA waits
            emit_q()
            emit_kv(0)
            R0 = begin_round(0, 0, *ROUNDS[0])
            do_kts(R0, range(0, 4))
            emit_kv(1)
            do_kts(R0, range(4, 8))
            emit_kv(2)
            do_kts(R0, range(8, 12))
            emit_kv(3)
            do_kts(R0, range(12, 16))
            finish_round(R0)
            R1 = begin_round(1, 0, *ROUNDS[1])
            do_kts(R1, range(16))
            finish_round(R1)
            R2 = begin_round(2, 1, *ROUNDS[0])
            do_kts(R2, range(16))
            finish_round(R2)
            R3 = begin_round(3, 1, *ROUNDS[1])
            do_kts(R3, range(16))
            finish_round(R3)
            emit_wo(1)

    nc.compile()
    return nc


def _host_tables(T):
    """Per-row-half bias tables in the final SBUF gather layout.

    Returns {r: sch int16 [128, 4*8448]}.
    Layout: partition p = 16*h2p + w2, free = (head, rdw 11, f 384) where
    f = 16*drh + w1, gathered value
    G[p,h,rdw,f] = bias_table[h, 4r+rdw, (7-h2p)+drh, 15+w1-w2].
    """
    T = np.asarray(T, dtype=np.float32)
    p = np.arange(128)
    h2p, w2 = p // 16, p % 16
    f = np.arange(384)
    drh, w1 = f // 16, f % 16
    rh = (7 - h2p)[:, None] + drh[None, :]          # [128, 384]
    rw = 15 + w1[None, :] - w2[:, None]             # [128, 384]
    out = {}
    for r in (0, 1):
        Twin = T[:, 4 * r:4 * r + 11]               # [8, 11, 31, 31]
        G = Twin[:, :, rh, rw]                      # [8, 11, 128, 384]
        G = np.ascontiguousarray(G.transpose(2, 0, 1, 3))  # [128, 8, 11, 384]
        sch = np.ascontiguousarray(
            np.round(A16 * (G - S_SHIFT) + B16)
            .reshape(128, -1).astype(np.int16)
        )
        out[r] = sch
    return out


def _host_inputs(x, Wq, Wk, Wv, Wo, bias_table):
    """Build the 8 per-core input maps."""
    import ml_dtypes

    bf = ml_dtypes.bfloat16
    x = np.asarray(x, dtype=np.float32)
    xf = np.ascontiguousarray(x.reshape(B, N, C))
    qsc = A16 / math.sqrt(HD)
    Wq_s = np.asarray(Wq, np.float32) * qsc
    Ws = {
        "Wq": np.ascontiguousarray(Wq_s.astype(bf)),
        "Wk": np.ascontiguousarray(np.asarray(Wk, np.float32).astype(bf)),
        "Wv": np.ascontiguousarray(np.asarray(Wv, np.float32).astype(bf)),
        "Wo": np.ascontiguousarray(np.asarray(Wo, np.float32).astype(bf)),
    }
    tables = _host_tables(bias_table)
    in_maps = []
    for c in range(8):
        b, r = c // 2, c % 2
        m = {
            "xT": np.ascontiguousarray(xf[b].T.astype(bf)),
            "xTq": np.ascontiguousarray(xf[b, QR * r:QR * (r + 1)].T.astype(bf)),
            "schT": tables[r],
            **Ws,
        }
        in_maps.append(m)
    return in_maps


def kernel(x, Wq, Wk, Wv, Wo, bias_table, _results_hook=None):
    global _NC
    if _NC is None:
        _NC = _build_nc()
    from concourse.bass_utils import run_bass_kernel_spmd

    in_maps = _host_inputs(x, Wq, Wk, Wv, Wo, bias_table)
    res = run_bass_kernel_spmd(_NC, in_maps, core_ids=list(range(8)))
    if _results_hook is not None:
        _results_hook(res)
    out = np.zeros((B, N, C), dtype=np.float32)
    for c in range(8):
        b, r = c // 2, c % 2
        out[b, QR * r:QR * (r + 1)] = res.results[c]["out"]
    D, H, W = 8, 16, 16
    return out.reshape(B, D, H, W, C)
